# revision 1
# baseline (speedup 1.0000x reference)
"""Multi-head attention (B=8, N=1024, C=1024, H=16) on 8 TRN2 NeuronCores.

Strategy: pure data parallelism — each core computes one batch element with
replicated weights (no collectives). Per-core single-head-dim layout:

  inputs (host-prepped, transposed so every matmul contracts on partitions):
    xT   [C, N]      = x[b].T
    wqkT [C, 2C_qk]  = in_proj_weight[0:2048].T     (q then k features)
    wvT  [C, C]      = in_proj_weight[2048:3072].T
    woT  [C, C]      = out_proj_weight.T            ((h,d) rows, co cols)
  phases on-device (all matmuls in float32r: full-rate fp32, ~1e-3 rounding):
    A: V natural [token, vfeat] per 65-wide head group w/ ones column
       (the ones column makes the PV matmul also produce softmax row-sums)
    B: qkT [feature, token] (transposed q/k for QK^T)
    C: per head h: S^T[key, query] = K_h^T.T @ Q_h^T; P = exp(S*scale);
       O^T[d+1, query] = V_hat.T @ P^T (row 64 = softmax sums);
       normalize via reciprocal + gpsimd partition_broadcast
    D: outT [co, token] = woT.T @ A^T + bias

Output: outT per core, host transposes back and stacks.
"""
import numpy as np

B, N, C = 8, 1024, 1024
H = 16
HD = C // H               # 64
SCALE = HD ** (-0.5)
NCORES = 8

_COMPILED = {}


def _build():
    import concourse.bass as bass
    import concourse.tile as tile
    from concourse import bacc, mybir

    F32 = mybir.dt.float32
    F32R = mybir.dt.float32r
    EXP = mybir.ActivationFunctionType.Exp

    nc = bacc.Bacc("TRN2", target_bir_lowering=False, debug=False)

    xT = nc.dram_tensor("xT", [C, N], F32R, kind="ExternalInput").ap()
    wqkT = nc.dram_tensor("wqkT", [C, 2 * C], F32R, kind="ExternalInput").ap()
    wvT = nc.dram_tensor("wvT", [C, C], F32R, kind="ExternalInput").ap()
    woT = nc.dram_tensor("woT", [C, C], F32R, kind="ExternalInput").ap()
    bqk = nc.dram_tensor("bqk", [128, 16], F32, kind="ExternalInput").ap()
    bv = nc.dram_tensor("bv", [1, C], F32R, kind="ExternalInput").ap()
    bo = nc.dram_tensor("bo", [128, 8], F32, kind="ExternalInput").ap()
    ones_col = nc.dram_tensor("ones_col", [128, 16], F32R, kind="ExternalInput").ap()
    ones_row = nc.dram_tensor("ones_row", [1, 512], F32R, kind="ExternalInput").ap()
    outT = nc.dram_tensor("outT", [C, N], F32, kind="ExternalOutput").ap()

    CB = C // 128      # 8 contraction blocks
    TB = N // 128      # 8 token blocks
    JB = 2 * C // 128  # 16 qk feature blocks
    VW = 65            # per-head V width (64 feats + ones col)

    with tile.TileContext(nc) as tc:
        with tc.tile_pool(name="misc", bufs=1) as pool_misc, \
             tc.tile_pool(name="V", bufs=1) as pool_V, \
             tc.tile_pool(name="qk", bufs=1) as pool_qk:

            bqk_sb = pool_misc.tile([128, 16], F32, tag="bqk")
            bv_sb = pool_misc.tile([1, C], F32R, tag="bv")
            bo_sb = pool_misc.tile([128, 8], F32, tag="bo")
            ones_sb = pool_misc.tile([1, 512], F32R, tag="ones")
            nc.sync.dma_start(bqk_sb[:, :], bqk)
            bv_rep = pool_misc.tile([128, C], F32, tag="bvrep")
            nc.sync.dma_start(bv_sb[:, :], bv)
            nc.sync.dma_start(bo_sb[:, :], bo)
            nc.sync.dma_start(ones_sb[:, :], ones_row)
            nc.gpsimd.partition_broadcast(bv_rep[:, :], bv_sb[0:1, :].bitcast(F32))

            V_sb = [pool_V.tile([128, H * VW], F32R, tag=f"V{tb}", name=f"V{tb}") for tb in range(TB)]
            qk_sb = [pool_qk.tile([128, N], F32R, tag=f"qk{jb}", name=f"qk{jb}") for jb in range(JB)]

            # ======== phases A (V natural) and B (qkT) ========
            with tc.tile_pool(name="x", bufs=1) as pool_x, \
                 tc.tile_pool(name="ps_proj", bufs=6, space="PSUM") as ps_proj:

                x_sb = [pool_x.tile([128, N], F32R, tag=f"x{cb}", name=f"x{cb}") for cb in range(CB)]

                # ---- A: V[token, vfeat] ----
                with tc.tile_pool(name="wv", bufs=1) as pool_wv:
                    wv_sb = [pool_wv.tile([128, C], F32R, tag=f"wv{cb}", name=f"wv{cb}") for cb in range(CB)]
                    # load order: x fully first (both A and B need it), then wv
                    for cb in range(CB):
                        for ch in range(2):
                            nc.sync.dma_start(
                                x_sb[cb][:, ch * 512:(ch + 1) * 512],
                                xT[cb * 128:(cb + 1) * 128, ch * 512:(ch + 1) * 512])
                    for cb in range(CB):
                        for ch in range(2):
                            eng = nc.sync if (cb + ch) % 2 == 1 else nc.scalar
                            eng.dma_start(
                                wv_sb[cb][:, ch * 512:(ch + 1) * 512],
                                wvT[cb * 128:(cb + 1) * 128, ch * 512:(ch + 1) * 512])
                    # ones columns of V_hat groups (only needed by phase C's PV)
                    for tb in range(TB):
                        nc.sync.dma_start(V_sb[tb][:, 64::VW], ones_col)
                    for tb in range(TB):
                        for vc in range(2):
                            ps = ps_proj.tile([128, 512], F32, tag="psA")
                            for cb in range(CB):
                                nc.tensor.matmul(
                                    ps[:, :],
                                    x_sb[cb][:, tb * 128:(tb + 1) * 128],
                                    wv_sb[cb][:, vc * 512:(vc + 1) * 512],
                                    start=(cb == 0), stop=(cb == CB - 1),
                                )
                            # scatter 8 heads x 64 cols into the 65-strided
                            # layout, adding the broadcast v bias
                            dst = V_sb[tb][:, vc * 8 * VW:(vc + 1) * 8 * VW]
                            dst3 = dst.rearrange("p (h d) -> p h d", h=8)[:, :, 0:64]
                            src3 = ps[:, :].rearrange("p (h d) -> p h d", h=8)
                            bv3 = bv_rep[:, vc * 512:(vc + 1) * 512].rearrange(
                                "p (h d) -> p h d", h=8)
                            nc.vector.tensor_add(dst3, src3, bv3)

                # ---- B: qkT[feature, token] ----
                with tc.tile_pool(name="wqk", bufs=12) as pool_wqk:
                    # k-feature half (jh=1) first so attention pairs can start
                    # as soon as their q block lands in the second half
                    for jh in (1, 0):  # stream wqk in two 1024-feature halves
                        wqk_sb = []
                        for cb in range(CB):
                            t = pool_wqk.tile([128, C], F32R, tag="wqk", name="wqk")
                            nc.sync.dma_start(
                                t[:, :],
                                wqkT[cb * 128:(cb + 1) * 128, jh * C:(jh + 1) * C],
                            )
                            wqk_sb.append(t)
                        for jbl in range(8):
                            jb = jh * 8 + jbl
                            for nch in range(2):
                                ps = ps_proj.tile([128, 512], F32, tag="psA")
                                for cb in range(CB):
                                    nc.tensor.matmul(
                                        ps[:, :],
                                        wqk_sb[cb][:, jbl * 128:(jbl + 1) * 128],
                                        x_sb[cb][:, nch * 512:(nch + 1) * 512],
                                        start=(cb == 0), stop=(cb == CB - 1),
                                    )
                                nc.vector.tensor_scalar(
                                    qk_sb[jb][:, nch * 512:(nch + 1) * 512], ps[:, :],
                                    bqk_sb[:, jb:jb + 1], None, mybir.AluOpType.add,
                                )

            # ======== phases C (attention) and D (out projection) ========
            # A^T reuses the q-feature qk tiles: block hp's q/k data is dead
            # once pair hp's S^T matmuls are done.
            A_sb = qk_sb[0:8]
            with tc.tile_pool(name="wo", bufs=1) as pool_wo:
                wo_sb = [pool_wo.tile([128, C], F32R, tag=f"wo{cb}", name=f"wo{cb}") for cb in range(CB)]
                for cb in range(CB):
                    nc.sync.dma_start(wo_sb[cb][:, :], woT[cb * 128:(cb + 1) * 128, :])

                with tc.tile_pool(name="PT", bufs=8) as pool_PT, \
                     tc.tile_pool(name="norm", bufs=2) as pool_norm, \
                     tc.tile_pool(name="ps_S", bufs=2, space="PSUM") as ps_S, \
                     tc.tile_pool(name="ps_O", bufs=2, space="PSUM") as ps_O:

                    # ---- flat skewed pipeline over 128 (pair, kb, ic) units.
                    # Each unit packs BOTH heads of the pair into one S tile:
                    # cols 0:512 = head h0, cols 512:1024 = head h1. The two QK
                    # matmuls land on complementary PE row groups (0-63 /
                    # 64-127) so they run concurrently and keep the array's
                    # activity monitor at full clock; one exp covers both.
                    units = [(hp, kb, ic)
                             for hp in range(8) for kb in range(TB)
                             for ic in range(2)]

                    s_ps_of = {}
                    pt_of = {}
                    o_ps_of = {}

                    def emit_qk(u):
                        hp, kb, ic = u
                        s_ps = ps_S.tile([128, N], F32, tag="S", name="S")
                        for hh in range(2):
                            r0, r1 = hh * 64, hh * 64 + 64
                            nc.tensor.matmul(
                                s_ps[:, hh * 512:(hh + 1) * 512],
                                qk_sb[8 + hp][r0:r1, kb * 128:(kb + 1) * 128],
                                qk_sb[hp][r0:r1, ic * 512:(ic + 1) * 512],
                                start=True, stop=True,
                            )
                        s_ps_of[u] = s_ps

                    def emit_exp(u):
                        p_t = pool_PT.tile([128, N], F32R, tag="pt", name="pt")
                        nc.scalar.activation(p_t[:, :], s_ps_of.pop(u)[:, :], EXP,
                                             scale=float(SCALE))
                        pt_of[u] = p_t

                    def emit_pv(u):
                        hp, kb, ic = u
                        p_t = pt_of.pop(u)
                        for hh in range(2):
                            h = 2 * hp + hh
                            if kb == 0 and ic == 0:
                                o_ps_of[h] = ps_O.tile([VW, N], F32, tag="O",
                                                       name="O")
                            nc.tensor.matmul(
                                o_ps_of[h][:, ic * 512:(ic + 1) * 512],
                                V_sb[kb][:, h * VW:(h + 1) * VW],
                                p_t[:, hh * 512:(hh + 1) * 512],
                                start=(kb == 0), stop=(kb == TB - 1),
                            )

                    def emit_norm(h):
                        hp, hh = h // 2, h % 2
                        o_ps = o_ps_of.pop(h)
                        # one copy to SBUF frees the PSUM bank immediately so
                        # the next pair's PV accumulation can begin
                        o_cp = pool_norm.tile([VW, N], F32, tag="ocp", name="ocp",
                                              bufs=2)
                        nc.vector.tensor_copy(o_cp[:, :], o_ps[:, :])
                        s128 = pool_norm.tile([128, 8], F32, tag="s128", name="s128")
                        nc.sync.dma_start(s128[:, :], o_cp[64:65, :])
                        r128 = pool_norm.tile([128, 8], F32, tag="r128", name="r128")
                        nc.vector.reciprocal(r128[:, :], s128[:, :])
                        r0t = pool_norm.tile([1, N], F32, tag="r0", name="r0")
                        nc.sync.dma_start(r0t[0:1, :], r128[:, :])
                        r_rep = pool_norm.tile([64, N], F32, tag="rrep", name="rrep")
                        nc.gpsimd.partition_broadcast(r_rep[:, :], r0t[0:1, :])
                        if hh == 0:
                            nc.vector.tensor_mul(
                                A_sb[hp][0:64, :], o_cp[0:64, :], r_rep[:, :])
                        else:
                            a_tmp = pool_norm.tile([64, N], F32R, tag="atmp",
                                                   name="atmp")
                            nc.vector.tensor_mul(
                                a_tmp[:, :], o_cp[0:64, :], r_rep[:, :])
                            for ch in range(4):
                                nc.sync.dma_start(
                                    A_sb[hp][64:128, ch * 256:(ch + 1) * 256],
                                    a_tmp[:, ch * 256:(ch + 1) * 256])

                    SKEW = 2
                    for j in range(SKEW):
                        emit_qk(units[j])
                    for i, u in enumerate(units):
                        emit_exp(u)
                        if i + SKEW < len(units):
                            emit_qk(units[i + SKEW])
                        emit_pv(u)
                        if u[1] == TB - 1 and u[2] == 1:
                            emit_norm(2 * u[0])
                            emit_norm(2 * u[0] + 1)

                with tc.tile_pool(name="outp", bufs=4) as pool_out, \
                         tc.tile_pool(name="ps_out", bufs=6, space="PSUM") as ps_out:
                        for cb in range(CB):
                            for nch in range(2):
                                ps = ps_out.tile([128, 512], F32, tag="po")
                                for hb in range(8):
                                    nc.tensor.matmul(
                                        ps[:, :],
                                        wo_sb[hb][:, cb * 128:(cb + 1) * 128],
                                        A_sb[hb][:, nch * 512:(nch + 1) * 512],
                                        start=(hb == 0), stop=(hb == 7),
                                    )
                                o_t = pool_out.tile([128, 512], F32, tag="ot")
                                nc.vector.tensor_scalar(
                                    o_t[:, :], ps[:, :], bo_sb[:, cb:cb + 1], None,
                                    mybir.AluOpType.add,
                                )
                                for sh in range(2):
                                    eng = nc.sync if (nch + sh) % 2 == 0 else nc.scalar
                                    eng.dma_start(
                                        outT[cb * 128:(cb + 1) * 128,
                                             nch * 512 + sh * 256:
                                             nch * 512 + (sh + 1) * 256],
                                        o_t[:, sh * 256:(sh + 1) * 256],
                                    )
    nc.compile()
    return nc


def _get_nc():
    if "nc" not in _COMPILED:
        _COMPILED["nc"] = _build()
    return _COMPILED["nc"]


def _run(x, in_proj_weight, in_proj_bias, out_proj_weight, out_proj_bias,
         trace=False):
    from concourse.bass_utils import run_bass_kernel_spmd

    nc = _get_nc()
    x = np.ascontiguousarray(np.asarray(x, dtype=np.float32))
    w_in = np.asarray(in_proj_weight, dtype=np.float32)
    b_in = np.asarray(in_proj_bias, dtype=np.float32)
    w_out = np.asarray(out_proj_weight, dtype=np.float32)
    b_out = np.asarray(out_proj_bias, dtype=np.float32)

    wqkT = np.ascontiguousarray(w_in[0:2 * C].T)          # [C, 2C]
    wvT = np.ascontiguousarray(w_in[2 * C:3 * C].T)       # [C, C]
    woT = np.ascontiguousarray(w_out.T)                   # [C, C]
    shared = {
        "wqkT": wqkT,
        "wvT": wvT,
        "woT": woT,
        "bqk": np.ascontiguousarray(b_in[0:2 * C].reshape(16, 128).T),
        "bv": np.ascontiguousarray(b_in[2 * C:3 * C])[None, :],
        "bo": np.ascontiguousarray(b_out.reshape(8, 128).T),
        "ones_col": np.ones((128, 16), dtype=np.float32),
        "ones_row": np.ones((1, 512), dtype=np.float32),
    }
    in_maps = []
    for c in range(NCORES):
        m = dict(shared)
        m["xT"] = np.ascontiguousarray(x[c].T)
        in_maps.append(m)

    res = run_bass_kernel_spmd(nc, in_maps, core_ids=list(range(NCORES)),
                               trace=trace)
    out = np.stack([
        np.ascontiguousarray(res.results[c]["outT"].T) for c in range(NCORES)
    ]).astype(np.float32)
    return out, res


def kernel(x, in_proj_weight, in_proj_bias, out_proj_weight, out_proj_bias):
    out, _ = _run(x, in_proj_weight, in_proj_bias, out_proj_weight,
                  out_proj_bias)
    return out



# revision 4
# speedup vs baseline: 1.1084x; 1.1084x over previous
"""Multi-head attention (B=8, N=1024, C=1024, H=16) on 8 TRN2 NeuronCores.

Strategy: pure data parallelism -- each core computes one batch element with
replicated weights (no collectives). All matmul operands are bf16 (PSUM
accumulation stays fp32), which halves HBM/SBUF traffic and keeps every
tensor resident so the phases can be globally interleaved.

Per-core layout (everything transposed so matmuls contract on partitions):
  xT  [C, N]   = x[b].T          wqT/wkT/wvT [C, C] = in_proj slices .T
  woT [C, C]   = out_proj_weight.T

Emission order = scheduler priority. The exp stream on the Scalar engine
(143us total) is the second-longest resource after the tensor engine
(~190us), so attention pairs are emitted early and projection / out-proj
matmuls are placed after each pair as fillers that soak up the tensor
engine whenever attention is waiting on exp:

  B(k0) B(q0) | [A(V) woven into pair0's units] | pair0 | B(k1) B(q1) |
  pair1 | B(k2) B(q2) | ... | pair7.ic0 | D(nch0) | pair7.ic1 | D(nch1)

Attention per pair hp (heads 2hp, 2hp+1), ic-major (query chunks of 512):
  QK: two row-group-concurrent matmuls (contraction 64) -> S [128k, 1024]
  exp: one ACT op per unit, bf16 out, scale folded in
  PV: V_hat [128k, 65] (64 dims + ones col -> softmax sums in row 64)
  norm: copy out of PSUM, reciprocal of row 64, gpsimd partition
        broadcast, multiply into A_sb (odd heads DMA-shifted to rows 64+)
  D:  out[cb, nch] = sum_hb woT_hb.T @ A_hb + bias
"""
import numpy as np

B, N, C = 8, 1024, 1024
H = 16
HD = C // H               # 64
SCALE = HD ** (-0.5)
NCORES = 8

_COMPILED = {}


def _build():
    import concourse.bass as bass
    import concourse.tile as tile
    from concourse import bacc, mybir

    F32 = mybir.dt.float32
    BF16 = mybir.dt.bfloat16
    EXP = mybir.ActivationFunctionType.Exp

    nc = bacc.Bacc("TRN2", target_bir_lowering=False, debug=False)

    xT = nc.dram_tensor("xT", [C, N], BF16, kind="ExternalInput").ap()
    wqT = nc.dram_tensor("wqT", [C, C], BF16, kind="ExternalInput").ap()
    wkT = nc.dram_tensor("wkT", [C, C], BF16, kind="ExternalInput").ap()
    wvT = nc.dram_tensor("wvT", [C, C], BF16, kind="ExternalInput").ap()
    woT = nc.dram_tensor("woT", [C, C], BF16, kind="ExternalInput").ap()
    bqk = nc.dram_tensor("bqk", [128, 16], F32, kind="ExternalInput").ap()
    bv = nc.dram_tensor("bv", [1, C], F32, kind="ExternalInput").ap()
    bo = nc.dram_tensor("bo", [128, 8], F32, kind="ExternalInput").ap()
    ones_col = nc.dram_tensor("ones_col", [128, 16], BF16, kind="ExternalInput").ap()
    outT = nc.dram_tensor("outT", [C, N], F32, kind="ExternalOutput").ap()

    CB = C // 128      # 8 contraction blocks
    TB = N // 128      # 8 token/key blocks
    VW = 65            # per-head V width (64 dims + ones col)

    with tile.TileContext(nc) as tc:
        with tc.tile_pool(name="misc", bufs=1) as pool_misc, \
             tc.tile_pool(name="w", bufs=1) as pool_w, \
             tc.tile_pool(name="qk", bufs=1) as pool_qk, \
             tc.tile_pool(name="V", bufs=1) as pool_V, \
             tc.tile_pool(name="A", bufs=1) as pool_A, \
             tc.tile_pool(name="PT", bufs=12) as pool_PT, \
             tc.tile_pool(name="norm", bufs=2) as pool_norm, \
             tc.tile_pool(name="outp", bufs=3) as pool_out, \
             tc.tile_pool(name="ps_S", bufs=2, space="PSUM") as ps_S, \
             tc.tile_pool(name="ps_O", bufs=2, space="PSUM") as ps_O, \
             tc.tile_pool(name="ps_fill", bufs=2, space="PSUM") as ps_fill:

            # ---------------- static tiles ----------------
            x_sb = [pool_w.tile([128, N], BF16, tag=f"x{cb}", name=f"x{cb}") for cb in range(CB)]
            wk_sb = [pool_w.tile([128, C], BF16, tag=f"wk{cb}", name=f"wk{cb}") for cb in range(CB)]
            wq_sb = [pool_w.tile([128, C], BF16, tag=f"wq{cb}", name=f"wq{cb}") for cb in range(CB)]
            wv_sb = [pool_w.tile([128, C], BF16, tag=f"wv{cb}", name=f"wv{cb}") for cb in range(CB)]
            wo_sb = [pool_w.tile([128, C], BF16, tag=f"wo{cb}", name=f"wo{cb}") for cb in range(CB)]
            # qk_sb[0..7] = q feature blocks, qk_sb[8..15] = k feature blocks
            qk_sb = [pool_qk.tile([128, N], BF16, tag=f"qk{jb}", name=f"qk{jb}") for jb in range(16)]
            V_sb = [pool_V.tile([128, H * VW], BF16, tag=f"V{tb}", name=f"V{tb}") for tb in range(TB)]
            A_sb = [pool_A.tile([128, N], BF16, tag=f"A{hp}", name=f"A{hp}") for hp in range(CB)]

            bqk_sb = pool_misc.tile([128, 16], F32, tag="bqk")
            bv_sb = pool_misc.tile([1, C], F32, tag="bv")
            bv_rep = pool_misc.tile([128, C], F32, tag="bvrep")
            bo_sb = pool_misc.tile([128, 8], F32, tag="bo")

            # ---------------- DMA loads (multi-queue) ----------------
            # sync: x first (everything contracts over it), then wv, wo.
            for cb in range(CB):
                nc.sync.dma_start(x_sb[cb][:, :], xT[cb * 128:(cb + 1) * 128, :])
            # scalar (ACT) queue is free until the first exp (~15us): k then
            # q weight halves so the first attention pair unblocks earliest.
            for cb in range(CB):
                nc.scalar.dma_start(wk_sb[cb][:, :], wkT[cb * 128:(cb + 1) * 128, :])
            for cb in range(CB):
                nc.scalar.dma_start(wq_sb[cb][:, :], wqT[cb * 128:(cb + 1) * 128, :])
            for cb in range(CB):
                nc.sync.dma_start(wv_sb[cb][:, :], wvT[cb * 128:(cb + 1) * 128, :])
            for tb in range(TB):
                nc.sync.dma_start(V_sb[tb][:, 64::VW], ones_col)
            for cb in range(CB):
                nc.sync.dma_start(wo_sb[cb][:, :], woT[cb * 128:(cb + 1) * 128, :])
            # biases on the gpsimd queue
            nc.gpsimd.dma_start(bqk_sb[:, :], bqk)
            nc.gpsimd.dma_start(bv_sb[:, :], bv)
            nc.gpsimd.dma_start(bo_sb[:, :], bo)
            nc.gpsimd.partition_broadcast(bv_rep[:, :], bv_sb[0:1, :])

            # ---------------- emission helpers ----------------
            def emit_B(jb):
                """qk feature block jb: qk_sb[jb] [128 feats, N tokens]."""
                w_sb = wq_sb if jb < 8 else wk_sb
                jbl = jb % 8
                for nch in range(2):
                    ps = ps_fill.tile([128, 512], F32, tag="fill")
                    for cb in range(CB):
                        nc.tensor.matmul(
                            ps[:, :],
                            w_sb[cb][:, jbl * 128:(jbl + 1) * 128],
                            x_sb[cb][:, nch * 512:(nch + 1) * 512],
                            start=(cb == 0), stop=(cb == CB - 1),
                        )
                    nc.vector.tensor_scalar(
                        qk_sb[jb][:, nch * 512:(nch + 1) * 512], ps[:, :],
                        bqk_sb[:, jb:jb + 1], None, mybir.AluOpType.add,
                    )

            def emit_A(tb):
                """V natural block tb: V_sb[tb] [128 tokens, 16*65]."""
                for vc in range(2):
                    ps = ps_fill.tile([128, 512], F32, tag="fill")
                    for cb in range(CB):
                        nc.tensor.matmul(
                            ps[:, :],
                            x_sb[cb][:, tb * 128:(tb + 1) * 128],
                            wv_sb[cb][:, vc * 512:(vc + 1) * 512],
                            start=(cb == 0), stop=(cb == CB - 1),
                        )
                    # scatter 8 heads x 64 dims into the 65-strided layout,
                    # adding the broadcast v bias
                    dst = V_sb[tb][:, vc * 8 * VW:(vc + 1) * 8 * VW]
                    dst3 = dst.rearrange("p (h d) -> p h d", h=8)[:, :, 0:64]
                    src3 = ps[:, :].rearrange("p (h d) -> p h d", h=8)
                    bv3 = bv_rep[:, vc * 512:(vc + 1) * 512].rearrange(
                        "p (h d) -> p h d", h=8)
                    nc.vector.tensor_add(dst3, src3, bv3)

            def emit_norm(hp, hh, ic, o_ps):
                """Normalize O (psum [65,512], row 64 = sums) into A_sb."""
                o_cp = pool_norm.tile([VW, 512], F32, tag="ocp", name="ocp")
                nc.vector.tensor_copy(o_cp[:, :], o_ps[:, :])  # frees bank
                # partition_broadcast sources partition 0 of the tile, so
                # stage the sums row there first
                sums0 = pool_norm.tile([1, 512], F32, tag="sums0", name="sums0")
                nc.sync.dma_start(sums0[:, :], o_cp[64:65, :])
                nc.vector.reciprocal(sums0[:, :], sums0[:, :])
                r_rep = pool_norm.tile([64, 512], F32, tag="rrep", name="rrep")
                nc.gpsimd.partition_broadcast(r_rep[:, :], sums0[0:1, :])
                if hh == 0:
                    nc.vector.tensor_mul(
                        A_sb[hp][0:64, ic * 512:(ic + 1) * 512],
                        o_cp[0:64, :], r_rep[:, :])
                else:
                    a_tmp = pool_norm.tile([64, 512], BF16, tag="atmp",
                                           name="atmp")
                    nc.vector.tensor_mul(a_tmp[:, :], o_cp[0:64, :], r_rep[:, :])
                    nc.gpsimd.dma_start(
                        A_sb[hp][64:128, ic * 512:(ic + 1) * 512], a_tmp[:, :])

            def emit_att_half(hp, ic, pre_units=None):
                """One query-chunk (512 cols) of attention pair hp.

                pre_units: optional callback(kb) emitted before unit kb's
                matmuls (used to weave V production into pair 0).
                """
                o_ps = {}
                for kb in range(TB):
                    if pre_units is not None:
                        pre_units(kb)
                    s_ps = ps_S.tile([128, N], F32, tag="S", name="S")
                    for hh in range(2):
                        r0, r1 = hh * 64, hh * 64 + 64
                        nc.tensor.matmul(
                            s_ps[:, hh * 512:(hh + 1) * 512],
                            qk_sb[8 + hp][r0:r1, kb * 128:(kb + 1) * 128],
                            qk_sb[hp][r0:r1, ic * 512:(ic + 1) * 512],
                            start=True, stop=True,
                        )
                    p_t = pool_PT.tile([128, N], BF16, tag="pt", name="pt")
                    nc.scalar.activation(p_t[:, :], s_ps[:, :], EXP,
                                         scale=float(SCALE))
                    for hh in range(2):
                        h = 2 * hp + hh
                        if kb == 0:
                            o_ps[hh] = ps_O.tile([VW, 512], F32, tag="O",
                                                 name="O")
                        nc.tensor.matmul(
                            o_ps[hh][:, :],
                            V_sb[kb][:, h * VW:(h + 1) * VW],
                            p_t[:, hh * 512:(hh + 1) * 512],
                            start=(kb == 0), stop=(kb == TB - 1),
                        )
                for hh in range(2):
                    emit_norm(hp, hh, ic, o_ps[hh])

            def emit_D(nch):
                """Out-projection for token chunk nch (needs all A_sb[:, nch])."""
                for cb in range(CB):
                    ps = ps_fill.tile([128, 512], F32, tag="fill")
                    for hb in range(CB):
                        nc.tensor.matmul(
                            ps[:, :],
                            wo_sb[hb][:, cb * 128:(cb + 1) * 128],
                            A_sb[hb][:, nch * 512:(nch + 1) * 512],
                            start=(hb == 0), stop=(hb == CB - 1),
                        )
                    o_t = pool_out.tile([128, 512], F32, tag="ot")
                    nc.vector.tensor_scalar(
                        o_t[:, :], ps[:, :], bo_sb[:, cb:cb + 1], None,
                        mybir.AluOpType.add,
                    )
                    nc.sync.dma_start(
                        outT[cb * 128:(cb + 1) * 128,
                             nch * 512:(nch + 1) * 512], o_t[:, :])

            # ---------------- global emission order ----------------
            emit_B(8)   # pair0 k features
            emit_B(0)   # pair0 q features
            # pair 0 with V production woven in just ahead of each key block
            emit_att_half(0, 0, pre_units=lambda kb: emit_A(kb))
            emit_att_half(0, 1)
            for hp in range(1, 8):
                emit_B(8 + hp)
                emit_B(hp)
                emit_att_half(hp, 0)
                if hp == 7:
                    emit_D(0)
                emit_att_half(hp, 1)
            emit_D(1)

    nc.compile()
    return nc


def _get_nc():
    if "nc" not in _COMPILED:
        _COMPILED["nc"] = _build()
    return _COMPILED["nc"]


def _run(x, in_proj_weight, in_proj_bias, out_proj_weight, out_proj_bias,
         trace=False):
    import ml_dtypes
    from concourse.bass_utils import run_bass_kernel_spmd

    BF = ml_dtypes.bfloat16
    nc = _get_nc()
    x = np.asarray(x, dtype=np.float32)
    w_in = np.asarray(in_proj_weight, dtype=np.float32)
    b_in = np.asarray(in_proj_bias, dtype=np.float32)
    w_out = np.asarray(out_proj_weight, dtype=np.float32)
    b_out = np.asarray(out_proj_bias, dtype=np.float32)

    shared = {
        "wqT": np.ascontiguousarray(w_in[0:C].T).astype(BF),
        "wkT": np.ascontiguousarray(w_in[C:2 * C].T).astype(BF),
        "wvT": np.ascontiguousarray(w_in[2 * C:3 * C].T).astype(BF),
        "woT": np.ascontiguousarray(w_out.T).astype(BF),
        "bqk": np.ascontiguousarray(b_in[0:2 * C].reshape(16, 128).T),
        "bv": np.ascontiguousarray(b_in[2 * C:3 * C])[None, :],
        "bo": np.ascontiguousarray(b_out.reshape(8, 128).T),
        "ones_col": np.ones((128, 16), dtype=BF),
    }
    in_maps = []
    for c in range(NCORES):
        m = dict(shared)
        m["xT"] = np.ascontiguousarray(x[c].T).astype(BF)
        in_maps.append(m)

    res = run_bass_kernel_spmd(nc, in_maps, core_ids=list(range(NCORES)),
                               trace=trace)
    out = np.stack([
        np.ascontiguousarray(res.results[c]["outT"].T) for c in range(NCORES)
    ]).astype(np.float32)
    return out, res


def kernel(x, in_proj_weight, in_proj_bias, out_proj_weight, out_proj_bias):
    out, _ = _run(x, in_proj_weight, in_proj_bias, out_proj_weight,
                  out_proj_bias)
    return out


# revision 11
# speedup vs baseline: 1.1165x; 1.0073x over previous
"""Multi-head attention (B=8, N=1024, C=1024, H=16) on 8 TRN2 NeuronCores.

Strategy: pure data parallelism -- each core computes one batch element with
replicated weights (no collectives). All matmul operands are bf16 (PSUM
accumulation stays fp32), which halves HBM/SBUF traffic and keeps every
tensor resident so the phases can be globally interleaved.

Per-core layout (everything transposed so matmuls contract on partitions):
  xT  [C, N]   = x[b].T          wqT/wkT/wvT [C, C] = in_proj slices .T
  woT [C, C]   = out_proj_weight.T

Emission order = scheduler priority. The exp stream on the Scalar engine
(143us total) is the second-longest resource after the tensor engine
(~190us), so attention pairs are emitted early and projection / out-proj
matmuls are placed after each pair as fillers that soak up the tensor
engine whenever attention is waiting on exp:

  B(k0) B(q0) | [A(V) woven into pair0's units] | pair0 | B(k1) B(q1) |
  pair1 | B(k2) B(q2) | ... | pair7.ic0 | D(nch0) | pair7.ic1 | D(nch1)

Attention per pair hp (heads 2hp, 2hp+1), ic-major (query chunks of 512):
  QK: two row-group-concurrent matmuls (contraction 64) -> S [128k, 1024]
  exp: one ACT op per unit, bf16 out, scale folded in
  PV: V_hat [128k, 65] (64 dims + ones col -> softmax sums in row 64)
  norm: copy out of PSUM, reciprocal of row 64, gpsimd partition
        broadcast, multiply into A_sb (odd heads DMA-shifted to rows 64+)
  D:  out[cb, nch] = sum_hb woT_hb.T @ A_hb + bias
"""
import numpy as np

B, N, C = 8, 1024, 1024
H = 16
HD = C // H               # 64
SCALE = HD ** (-0.5)
NCORES = 8

_COMPILED = {}


def _build():
    import concourse.bass as bass
    import concourse.tile as tile
    from concourse import bacc, mybir

    F32 = mybir.dt.float32
    BF16 = mybir.dt.bfloat16
    EXP = mybir.ActivationFunctionType.Exp

    nc = bacc.Bacc("TRN2", target_bir_lowering=False, debug=False)

    xT = nc.dram_tensor("xT", [C, N], BF16, kind="ExternalInput").ap()
    wqT = nc.dram_tensor("wqT", [C, C], BF16, kind="ExternalInput").ap()
    wkT = nc.dram_tensor("wkT", [C, C], BF16, kind="ExternalInput").ap()
    wvT = nc.dram_tensor("wvT", [C, C], BF16, kind="ExternalInput").ap()
    woT = nc.dram_tensor("woT", [C, C], BF16, kind="ExternalInput").ap()
    bqk = nc.dram_tensor("bqk", [128, 16], F32, kind="ExternalInput").ap()
    bv = nc.dram_tensor("bv", [1, C], F32, kind="ExternalInput").ap()
    bo = nc.dram_tensor("bo", [128, 8], F32, kind="ExternalInput").ap()
    ones_col = nc.dram_tensor("ones_col", [128, 16], BF16, kind="ExternalInput").ap()
    outT = nc.dram_tensor("outT", [C, N], F32, kind="ExternalOutput").ap()

    CB = C // 128      # 8 contraction blocks
    TB = N // 128      # 8 token/key blocks
    VW = 65            # per-head V width (64 dims + ones col)

    with tile.TileContext(nc) as tc:
        with tc.tile_pool(name="misc", bufs=1) as pool_misc, \
             tc.tile_pool(name="w", bufs=1) as pool_w, \
             tc.tile_pool(name="qk", bufs=1) as pool_qk, \
             tc.tile_pool(name="V", bufs=1) as pool_V, \
             tc.tile_pool(name="A", bufs=1) as pool_A, \
             tc.tile_pool(name="PT", bufs=12) as pool_PT, \
             tc.tile_pool(name="norm", bufs=2) as pool_norm, \
             tc.tile_pool(name="outp", bufs=3) as pool_out, \
             tc.tile_pool(name="ps_S", bufs=2, space="PSUM") as ps_S, \
             tc.tile_pool(name="ps_O", bufs=2, space="PSUM") as ps_O, \
             tc.tile_pool(name="ps_fill", bufs=2, space="PSUM") as ps_fill:

            # ---------------- static tiles ----------------
            x_sb = [pool_w.tile([128, N], BF16, tag=f"x{cb}", name=f"x{cb}") for cb in range(CB)]
            wk_sb = [pool_w.tile([128, C], BF16, tag=f"wk{cb}", name=f"wk{cb}") for cb in range(CB)]
            wq_sb = [pool_w.tile([128, C], BF16, tag=f"wq{cb}", name=f"wq{cb}") for cb in range(CB)]
            wv_sb = [pool_w.tile([128, C], BF16, tag=f"wv{cb}", name=f"wv{cb}") for cb in range(CB)]
            wo_sb = [pool_w.tile([128, C], BF16, tag=f"wo{cb}", name=f"wo{cb}") for cb in range(CB)]
            # qk_sb[0..7] = q feature blocks, qk_sb[8..15] = k feature blocks
            qk_sb = [pool_qk.tile([128, N], BF16, tag=f"qk{jb}", name=f"qk{jb}") for jb in range(16)]
            V_sb = [pool_V.tile([128, H * VW], BF16, tag=f"V{tb}", name=f"V{tb}") for tb in range(TB)]
            A_sb = [pool_A.tile([128, N], BF16, tag=f"A{hp}", name=f"A{hp}") for hp in range(CB)]

            bqk_sb = pool_misc.tile([128, 16], F32, tag="bqk")
            bv_sb = pool_misc.tile([1, C], F32, tag="bv")
            bv_rep = pool_misc.tile([128, C], F32, tag="bvrep")
            bo_sb = pool_misc.tile([128, 8], F32, tag="bo")

            # ---------------- DMA loads (multi-queue) ----------------
            # HBM is the startup wall (~350 GB/s/core): order streams by
            # first-use.  sync: x then wq; scalar (free until first exp):
            # wv then wk; gpsimd: biases + wo (not needed until ~150us).
            for cb in range(CB):
                nc.sync.dma_start(x_sb[cb][:, :], xT[cb * 128:(cb + 1) * 128, :])
            for cb in range(CB):
                nc.scalar.dma_start(wv_sb[cb][:, :], wvT[cb * 128:(cb + 1) * 128, :])
            for cb in range(CB):
                nc.scalar.dma_start(wk_sb[cb][:, :], wkT[cb * 128:(cb + 1) * 128, :])
            for cb in range(CB):
                nc.sync.dma_start(wq_sb[cb][:, :], wqT[cb * 128:(cb + 1) * 128, :])
            for tb in range(TB):
                nc.sync.dma_start(V_sb[tb][:, 64::VW], ones_col)
            # biases + wo on the gpsimd queue
            nc.gpsimd.dma_start(bqk_sb[:, :], bqk)
            nc.gpsimd.dma_start(bv_sb[:, :], bv)
            nc.gpsimd.dma_start(bo_sb[:, :], bo)
            nc.gpsimd.partition_broadcast(bv_rep[:, :], bv_sb[0:1, :])
            for cb in range(CB):
                nc.gpsimd.dma_start(wo_sb[cb][:, :], woT[cb * 128:(cb + 1) * 128, :])

            # ---------------- emission helpers ----------------
            def emit_B_chunk(jb, nch):
                """Half of a qk feature block: qk_sb[jb][:, nch*512:]."""
                w_sb = wq_sb if jb < 8 else wk_sb
                jbl = jb % 8
                ps = ps_fill.tile([128, 512], F32, tag="fill")
                for cb in range(CB):
                    nc.tensor.matmul(
                        ps[:, :],
                        w_sb[cb][:, jbl * 128:(jbl + 1) * 128],
                        x_sb[cb][:, nch * 512:(nch + 1) * 512],
                        start=(cb == 0), stop=(cb == CB - 1),
                    )
                nc.vector.tensor_scalar(
                    qk_sb[jb][:, nch * 512:(nch + 1) * 512], ps[:, :],
                    bqk_sb[:, jb:jb + 1], None, mybir.AluOpType.add,
                )

            def emit_B(jb):
                emit_B_chunk(jb, 0)
                emit_B_chunk(jb, 1)

            def emit_A(tb):
                """V natural block tb: V_sb[tb] [128 tokens, 16*65]."""
                for vc in range(2):
                    ps = ps_fill.tile([128, 512], F32, tag="fill")
                    for cb in range(CB):
                        nc.tensor.matmul(
                            ps[:, :],
                            x_sb[cb][:, tb * 128:(tb + 1) * 128],
                            wv_sb[cb][:, vc * 512:(vc + 1) * 512],
                            start=(cb == 0), stop=(cb == CB - 1),
                        )
                    # scatter 8 heads x 64 dims into the 65-strided layout,
                    # adding the broadcast v bias
                    dst = V_sb[tb][:, vc * 8 * VW:(vc + 1) * 8 * VW]
                    dst3 = dst.rearrange("p (h d) -> p h d", h=8)[:, :, 0:64]
                    src3 = ps[:, :].rearrange("p (h d) -> p h d", h=8)
                    bv3 = bv_rep[:, vc * 512:(vc + 1) * 512].rearrange(
                        "p (h d) -> p h d", h=8)
                    nc.vector.tensor_add(dst3, src3, bv3)

            def emit_norm(hp, hh, ic, o_ps):
                """Normalize O (psum [65,512], row 64 = sums) into A_sb."""
                o_cp = pool_norm.tile([VW, 512], F32, tag="ocp", name="ocp")
                nc.vector.tensor_copy(o_cp[:, :], o_ps[:, :])  # frees bank
                # single-partition reciprocal is ~6 cyc/elem on one DVE lane
                # (3.3us for 512) -- reshape to [128,4] so it runs wide, then
                # back to partition 0 where partition_broadcast sources from
                s128 = pool_norm.tile([128, 4], F32, tag="s128", name="s128")
                nc.sync.dma_start(s128[:, :], o_cp[64:65, :])
                nc.vector.reciprocal(s128[:, :], s128[:, :])
                sums0 = pool_norm.tile([1, 512], F32, tag="sums0", name="sums0")
                nc.sync.dma_start(sums0[:, :], s128[:, :])
                r_rep = pool_norm.tile([64, 512], F32, tag="rrep", name="rrep")
                nc.gpsimd.partition_broadcast(r_rep[:, :], sums0[0:1, :])
                if hh == 0:
                    nc.vector.tensor_mul(
                        A_sb[hp][0:64, ic * 512:(ic + 1) * 512],
                        o_cp[0:64, :], r_rep[:, :])
                else:
                    a_tmp = pool_norm.tile([64, 512], BF16, tag="atmp",
                                           name="atmp")
                    nc.vector.tensor_mul(a_tmp[:, :], o_cp[0:64, :], r_rep[:, :])
                    nc.gpsimd.dma_start(
                        A_sb[hp][64:128, ic * 512:(ic + 1) * 512], a_tmp[:, :])

            def emit_att_half(hp, ic, mid=None):
                """One query-chunk (512 cols) of attention pair hp.

                mid: optional callback(kb) emitted between unit kb's exp and
                its PV matmuls -- filler work woven at the point where it
                cannot delay the exp stream but precedes (program-order) any
                consumer that needs it.
                """
                o_ps = {}
                for kb in range(TB):
                    s_ps = ps_S.tile([128, N], F32, tag="S", name="S")
                    for hh in range(2):
                        r0, r1 = hh * 64, hh * 64 + 64
                        nc.tensor.matmul(
                            s_ps[:, hh * 512:(hh + 1) * 512],
                            qk_sb[8 + hp][r0:r1, kb * 128:(kb + 1) * 128],
                            qk_sb[hp][r0:r1, ic * 512:(ic + 1) * 512],
                            start=True, stop=True,
                        )
                    p_t = pool_PT.tile([128, N], BF16, tag="pt", name="pt")
                    nc.scalar.activation(p_t[:, :], s_ps[:, :], EXP,
                                         scale=float(SCALE))
                    if mid is not None:
                        mid(kb)
                    for hh in range(2):
                        h = 2 * hp + hh
                        if kb == 0:
                            o_ps[hh] = ps_O.tile([VW, 512], F32, tag="O",
                                                 name="O")
                        nc.tensor.matmul(
                            o_ps[hh][:, :],
                            V_sb[kb][:, h * VW:(h + 1) * VW],
                            p_t[:, hh * 512:(hh + 1) * 512],
                            start=(kb == 0), stop=(kb == TB - 1),
                        )
                for hh in range(2):
                    emit_norm(hp, hh, ic, o_ps[hh])

            def emit_D(nch):
                """Out-projection for token chunk nch (needs all A_sb[:, nch])."""
                for cb in range(CB):
                    ps = ps_fill.tile([128, 512], F32, tag="fill")
                    for hb in range(CB):
                        nc.tensor.matmul(
                            ps[:, :],
                            wo_sb[hb][:, cb * 128:(cb + 1) * 128],
                            A_sb[hb][:, nch * 512:(nch + 1) * 512],
                            start=(hb == 0), stop=(hb == CB - 1),
                        )
                    o_t = pool_out.tile([128, 512], F32, tag="ot")
                    nc.vector.tensor_scalar(
                        o_t[:, :], ps[:, :], bo_sb[:, cb:cb + 1], None,
                        mybir.AluOpType.add,
                    )
                    nc.sync.dma_start(
                        outT[cb * 128:(cb + 1) * 128,
                             nch * 512:(nch + 1) * 512], o_t[:, :])

            # ---------------- global emission order ----------------
            # Emission order is BOTH program order (dataflow: a reader
            # emitted before a writer sees stale data) and scheduler
            # priority.  Fillers are woven post-exp inside attention units:
            # there they cannot stall the exp stream but still precede
            # (in program order) everything that consumes them.
            emit_B(8)   # pair0 k features
            emit_B(0)   # pair0 q features
            # V blocks 0-1 fill the DMA-bound startup window before the
            # first QK is ready; blocks 2-7 are woven into pair0.ic0.
            emit_A(0)
            emit_A(1)
            emit_att_half(0, 0, mid=lambda kb: emit_A(kb + 2) if kb < 6 else None)
            # pair1 features woven into pair0.ic1
            ic1_fill = {1: lambda: emit_B_chunk(9, 0), 3: lambda: emit_B_chunk(9, 1),
                        5: lambda: emit_B_chunk(1, 0), 7: lambda: emit_B_chunk(1, 1)}
            emit_att_half(0, 1, mid=lambda kb: ic1_fill[kb]() if kb in ic1_fill else None)
            for hp in range(1, 8):
                if hp < 7:
                    # next pair's features woven across this pair's 16 units
                    nj, = [8 + hp + 1],
                    fills = {(0, 2): (8 + hp + 1, 0), (0, 6): (8 + hp + 1, 1),
                             (1, 2): (hp + 1, 0), (1, 6): (hp + 1, 1)}
                    emit_att_half(hp, 0, mid=lambda kb, f=fills, h=hp:
                                  emit_B_chunk(*f[(0, kb)]) if (0, kb) in f else None)
                    emit_att_half(hp, 1, mid=lambda kb, f=fills, h=hp:
                                  emit_B_chunk(*f[(1, kb)]) if (1, kb) in f else None)
                else:
                    emit_att_half(hp, 0)
                    emit_D(0)
                    emit_att_half(hp, 1)
            emit_D(1)

    nc.compile()
    return nc


def _get_nc():
    if "nc" not in _COMPILED:
        _COMPILED["nc"] = _build()
    return _COMPILED["nc"]


def _run(x, in_proj_weight, in_proj_bias, out_proj_weight, out_proj_bias,
         trace=False):
    import ml_dtypes
    from concourse.bass_utils import run_bass_kernel_spmd

    BF = ml_dtypes.bfloat16
    nc = _get_nc()
    x = np.asarray(x, dtype=np.float32)
    w_in = np.asarray(in_proj_weight, dtype=np.float32)
    b_in = np.asarray(in_proj_bias, dtype=np.float32)
    w_out = np.asarray(out_proj_weight, dtype=np.float32)
    b_out = np.asarray(out_proj_bias, dtype=np.float32)

    shared = {
        "wqT": np.ascontiguousarray(w_in[0:C].T).astype(BF),
        "wkT": np.ascontiguousarray(w_in[C:2 * C].T).astype(BF),
        "wvT": np.ascontiguousarray(w_in[2 * C:3 * C].T).astype(BF),
        "woT": np.ascontiguousarray(w_out.T).astype(BF),
        "bqk": np.ascontiguousarray(b_in[0:2 * C].reshape(16, 128).T),
        "bv": np.ascontiguousarray(b_in[2 * C:3 * C])[None, :],
        "bo": np.ascontiguousarray(b_out.reshape(8, 128).T),
        "ones_col": np.ones((128, 16), dtype=BF),
    }
    in_maps = []
    for c in range(NCORES):
        m = dict(shared)
        m["xT"] = np.ascontiguousarray(x[c].T).astype(BF)
        in_maps.append(m)

    res = run_bass_kernel_spmd(nc, in_maps, core_ids=list(range(NCORES)),
                               trace=trace)
    out = np.stack([
        np.ascontiguousarray(res.results[c]["outT"].T) for c in range(NCORES)
    ]).astype(np.float32)
    return out, res


def kernel(x, in_proj_weight, in_proj_bias, out_proj_weight, out_proj_bias):
    out, _ = _run(x, in_proj_weight, in_proj_bias, out_proj_weight,
                  out_proj_bias)
    return out
